# revision 14
# baseline (speedup 1.0000x reference)
"""BiLSTM Trainium2 kernel — transposed-domain recurrence.

Problem: B=32, T=512, I=512, H=512 bidirectional LSTM (torch gate order
i,f,g,o; shared Wx/Wh/bx/bh across directions; backward outputs stacked in
processing order).

Sharding: 8 cores = 2 directions x 4 batch groups of 8. Backward cores get
x time-reversed on the host so every core runs the same forward program.

Device program (per core, BL=8 batch rows):
  The whole recurrence runs transposed: gates live as gatesT [2048 gate dims
  (16 chunks of 128 partitions), 8 batch] in a single PSUM tile [128, 16*8].
  Each chunk accumulates 9 bf16 matmuls: 1 bias (K=1), 4 x-chunks and 4
  h-chunks, all with stationary 128x128 weight blocks and an 8-column moving
  operand (8 cycles each). The x/bias matmuls for step t+1 run during step
  t's epilogue, so only the 64 h-matmuls sit on the serial chain.

  Everything is sigmoid: the g gate's weights/bias are pre-scaled x2 on the
  host so tanh(a) = 2*sigmoid(2a)-1, and Wh is pre-doubled so the carried
  state is hhat = h/2. Epilogue per step (tiles [128, 32] = 4 h-chunks x 8):
      sgi  = sigmoid(gatesT[g,i])      ACT  [128, 64]
      sfo  = sigmoid(gatesT[f,o])      ACT  [128, 64]
      x1   = (ghat - 0.5) * i          DVE  scalar_tensor_tensor
      fc   = f * c                     Pool (parallel with x1)
      c    = 2*x1 + fc                 DVE  scalar_tensor_tensor
      chat = sigmoid(2c)               ACT  (tanh(c) = 2*chat - 1)
      hhat = (chat - 0.5) * o -> bf16  DVE  (= h/2; y = 2*hhat on host)
  hhat is written straight into a [128, 16*32] window buffer that is both
  the next steps' matmul rhs and the y DMA source (one DMA per 16 steps).
"""

import numpy as np

B, T, I, H = 32, 512, 512, 512
G4 = 4 * H            # 2048 gate width
BL = 8                # batch rows per core
NCH = 16              # gate chunks of 128
YW = 16               # steps per y DMA window
FILL_N = 0            # PE filler matmuls per step (keep max p-state)
FILL_AP = 256

_COMPILED = {}


def _build_program(t_steps: int):
    import concourse.bass as bass
    import concourse.tile as tile
    from concourse import bacc, mybir

    dt = mybir.dt
    f32 = dt.float32
    bf16 = dt.bfloat16
    alu = mybir.AluOpType
    sigf = mybir.ActivationFunctionType.Sigmoid

    nc = bacc.Bacc("TRN2", target_bir_lowering=False, debug=False)

    xT_d = nc.declare_dram_parameter("xT", [I, t_steps * BL], bf16, isOutput=False)
    WxT_d = nc.declare_dram_parameter("WxT", [I, G4], bf16, isOutput=False)
    WhT_d = nc.declare_dram_parameter("WhT", [H, G4], bf16, isOutput=False)
    bT_d = nc.declare_dram_parameter("bT", [1, G4], bf16, isOutput=False)
    y_d = nc.declare_dram_parameter("y", [128, t_steps * 4 * BL], bf16, isOutput=True)

    CW = 4 * BL  # 32 free columns per [128, 32] epilogue tile

    with tile.TileContext(nc) as tc:
        with (
            tc.tile_pool(name="const", bufs=1) as const_pool,
            tc.tile_pool(name="ybuf", bufs=2) as ybuf_pool,
            tc.tile_pool(name="sig", bufs=4) as sig_pool,
            tc.tile_pool(name="ep", bufs=4) as ep_pool,
            tc.tile_pool(name="gates", bufs=2, space="PSUM") as gates_pool,
        ):
            # ---- constants ----
            whT = []
            wxT = []
            for k in range(4):
                t_ = const_pool.tile([128, G4], bf16, tag=f"whT{k}", name=f"whT{k}")
                nc.sync.dma_start(out=t_, in_=WhT_d[k * 128 : (k + 1) * 128, :])
                whT.append(t_)
            for k in range(4):
                t_ = const_pool.tile([128, G4], bf16, tag=f"wxT{k}", name=f"wxT{k}")
                nc.sync.dma_start(out=t_, in_=WxT_d[k * 128 : (k + 1) * 128, :])
                wxT.append(t_)
            xT = []
            for k in range(4):
                t_ = const_pool.tile([128, t_steps * BL], bf16, tag=f"xT{k}", name=f"xT{k}")
                nc.sync.dma_start(out=t_, in_=xT_d[k * 128 : (k + 1) * 128, :])
                xT.append(t_)
            bT = const_pool.tile([1, G4], bf16, tag="bT")
            nc.sync.dma_start(out=bT, in_=bT_d[:, :])
            ones = const_pool.tile([1, BL], bf16, tag="ones")
            nc.vector.memset(ones, 1.0)

            # chunk ch covers gate dims [ch*128, (ch+1)*128) in device order
            # [f(0..3) g(4..7) i(8..11) o(12..15)]
            def csl(ch):
                return slice(ch * BL, (ch + 1) * BL)

            # PSUM zero-region rule: one start and one stop per bank per
            # step. The first matmul of a step (bias ch 0) starts the group
            # (hardware zeroes the whole bank lazily); the last matmul of
            # the step stops it.
            def emit_bias_x(gates, t):
                stop = t == 0  # at t==0 there is no h part; stop here
                for ch in range(NCH):
                    out = gates[:, csl(ch)]
                    nc.tensor.matmul(
                        out,
                        lhsT=bT[:, ch * 128 : (ch + 1) * 128],
                        rhs=ones[:, :],
                        start=ch == 0,
                        stop=False,
                    )
                    for k in range(4):
                        nc.tensor.matmul(
                            out,
                            lhsT=wxT[k][:, ch * 128 : (ch + 1) * 128],
                            rhs=xT[k][:, t * BL : (t + 1) * BL],
                            start=False,
                            stop=stop and ch == NCH - 1 and k == 3,
                        )

            def emit_h(gates, hhat):
                for ch in range(NCH):
                    out = gates[:, csl(ch)]
                    for k in range(4):
                        nc.tensor.matmul(
                            out,
                            lhsT=whT[k][:, ch * 128 : (ch + 1) * 128],
                            rhs=hhat[:, k * BL : (k + 1) * BL],
                            start=False,
                            stop=ch == NCH - 1 and k == 3,
                        )

            # ---- prologue ----
            # filler matmuls keep the PE busy through the epilogue so the
            # tensor engine never drops out of its max p-state.
            fill_ps = gates_pool.tile([128, 512], f32, tag="fill", name="fill_ps")

            def emit_fillers(n, ap=256):
                ap = min(ap, t_steps * BL)
                for _ in range(n):
                    nc.tensor.matmul(
                        fill_ps[:, 0:ap],
                        lhsT=whT[0][:, 0:128],
                        rhs=xT[0][:, 0:ap],
                        start=True,
                        stop=True,
                    )

            c = ep_pool.tile([128, CW], f32, tag="c")
            nc.vector.memset(c, 0.0)

            # full-bank tiles (2KB/partition) so each step owns its zero regions
            gates = gates_pool.tile([128, 512], f32, tag="gates", name="gates0")
            emit_bias_x(gates, 0)

            ybuf = ybuf_pool.tile([128, YW * CW], bf16, tag="ybuf", name="ybuf0")
            hhat = None

            # ---- main loop ----
            for t in range(t_steps):
                if t > 0:
                    emit_h(gates, hhat)

                # queue next step's bias+x matmuls right behind this step's
                # h-mms, BEFORE emitting this step's ACT reads: the scheduler
                # binds the PSUM WAR dependency to the reads emitted so far
                # (step t-1's, long done), not step t's.
                if t + 1 < t_steps:
                    gates_next = gates_pool.tile(
                        [128, 512], f32, tag="gates", name=f"gates{t + 1}"
                    )
                    emit_bias_x(gates_next, t + 1)
                else:
                    gates_next = None
                emit_fillers(FILL_N, FILL_AP)

                # One ACT covers f,g,i (the chain-critical gates); o follows.
                # Consecutive same-engine ACTs get chained on each other's
                # completion sems by the scheduler, so fewer ACTs = less
                # serial latency.
                sfgi = sig_pool.tile([128, 3 * CW], f32, tag="sfgi", name=f"sfgi{t}")
                so = sig_pool.tile([128, CW], f32, tag="so", name=f"so{t}")
                nc.scalar.activation(sfgi, gates[:, 0 : 3 * CW], sigf)
                nc.scalar.activation(so, gates[:, 3 * CW : 4 * CW], sigf)

                x1 = ep_pool.tile([128, CW], f32, tag="x1")
                fc = ep_pool.tile([128, CW], f32, tag="fc")
                cn = ep_pool.tile([128, CW], f32, tag="c")
                chat = ep_pool.tile([128, CW], f32, tag="chat")
                # x1/fc/cn back-to-back on DVE: same-queue FIFO needs no sems
                # x1 = (ghat - 0.5) * i
                nc.vector.scalar_tensor_tensor(
                    x1, sfgi[:, CW : 2 * CW], 0.5, sfgi[:, 2 * CW : 3 * CW],
                    op0=alu.subtract, op1=alu.mult,
                )
                # fc = f * c
                nc.vector.tensor_tensor(fc, sfgi[:, 0:CW], c, op=alu.mult)
                # c = 2*x1 + fc
                nc.vector.scalar_tensor_tensor(
                    cn, x1, 2.0, fc, op0=alu.mult, op1=alu.add,
                )
                # chat = sigmoid(2c)      on ACT
                nc.scalar.activation(chat, cn, sigf, scale=2.0)
                # hhat = (chat-0.5) * o   on DVE, bf16, into the y window buf
                j = t % YW
                hn = ybuf[:, j * CW : (j + 1) * CW]
                nc.vector.scalar_tensor_tensor(
                    hn, chat, 0.5, so,
                    op0=alu.subtract, op1=alu.mult,
                )

                if j == YW - 1:
                    w = t // YW
                    nc.sync.dma_start(
                        out=y_d[:, w * YW * CW : (w + 1) * YW * CW], in_=ybuf
                    )
                    if t + 1 < t_steps:
                        ybuf = ybuf_pool.tile(
                            [128, YW * CW], bf16, tag="ybuf", name=f"ybuf{t + 1}"
                        )

                c = cn
                hhat = hn
                gates = gates_next

    nc.compile()
    return nc


def _get_program(t_steps: int):
    if t_steps not in _COMPILED:
        _COMPILED[t_steps] = _build_program(t_steps)
    return _COMPILED[t_steps]


# gate permutation: device order [f, g, i, o] from torch order [i, f, g, o]
_PERM = np.concatenate(
    [np.arange(512, 1024), np.arange(1024, 1536), np.arange(0, 512), np.arange(1536, 2048)]
)


def _host_prep(x, Wx, bx, Wh, bh, t_steps):
    import ml_dtypes

    bf = ml_dtypes.bfloat16
    Wx_p = Wx[_PERM].astype(np.float32).copy()
    Wh_p = Wh[_PERM].astype(np.float32).copy()
    b_p = (bx + bh)[_PERM].astype(np.float32).copy()
    # g rows (device chunks 4..7) carry 2x so sigmoid(2a) = (tanh(a)+1)/2
    Wx_p[512:1024] *= 2.0
    b_p[512:1024] *= 2.0
    Wh_p[512:1024] *= 2.0
    # carried state is hhat = h/2 -> double all Wh columns' effect
    Wh_p *= 2.0

    WxT = np.ascontiguousarray(Wx_p.T).astype(bf)
    WhT = np.ascontiguousarray(Wh_p.T).astype(bf)
    bT = np.ascontiguousarray(b_p.reshape(1, G4)).astype(bf)

    in_maps = []
    for core in range(8):
        d, g = divmod(core, 4)
        xc = x[g * BL : (g + 1) * BL, :t_steps]
        if d == 1:
            xc = xc[:, ::-1]
        xT = np.ascontiguousarray(xc.transpose(2, 1, 0).reshape(I, t_steps * BL))
        in_maps.append(
            {"xT": xT.astype(bf), "WxT": WxT, "WhT": WhT, "bT": bT}
        )
    return in_maps


def _unshard_y(y, t_steps):
    # y [128, t*4*BL] bf16 -> h [BL, t, H]; h = 2*hhat
    yh = 2.0 * np.asarray(y, dtype=np.float32).reshape(128, t_steps, 4, BL)
    return yh.transpose(3, 1, 2, 0).reshape(BL, t_steps, H)


def kernel(x, Wx, bx, Wh, bh):
    from concourse.bass_utils import run_bass_kernel_spmd

    x = np.asarray(x, dtype=np.float32)
    Wx = np.asarray(Wx, dtype=np.float32)
    bx = np.asarray(bx, dtype=np.float32)
    Wh = np.asarray(Wh, dtype=np.float32)
    bh = np.asarray(bh, dtype=np.float32)
    nc = _get_program(T)
    in_maps = _host_prep(x, Wx, bx, Wh, bh, T)
    res = run_bass_kernel_spmd(nc, in_maps, list(range(8)))
    out = np.empty((B, T, 2 * H), dtype=np.float32)
    for core in range(8):
        d, g = divmod(core, 4)
        yh = _unshard_y(res.results[core]["y"], T)
        out[g * BL : (g + 1) * BL, :, d * H : (d + 1) * H] = yh
    return out


def _np_lstm(x, Wx, bx, Wh, bh):
    """Single-direction numpy reference for self-test (forward order)."""
    b_, t_, _ = x.shape
    h = np.zeros((b_, H), np.float32)
    c = np.zeros((b_, H), np.float32)
    gx = x @ Wx.T + bx
    ys = []
    for t in range(t_):
        gates = gx[:, t] + h @ Wh.T + bh
        i_g, f_g, g_g, o_g = np.split(gates, 4, axis=1)
        i_t = 1 / (1 + np.exp(-i_g))
        f_t = 1 / (1 + np.exp(-f_g))
        g_t = np.tanh(g_g)
        o_t = 1 / (1 + np.exp(-o_g))
        c = c * f_t + i_t * g_t
        h = o_t * np.tanh(c)
        ys.append(h)
    return np.stack(ys, 1)


def _selftest(t_steps=16):
    from concourse.bass_interp import CoreSim

    rng = np.random.default_rng(0)
    s = 1.0 / np.sqrt(H)
    x = rng.standard_normal((B, T, I), dtype=np.float32)
    Wx = rng.standard_normal((G4, I), dtype=np.float32) * s
    bx = rng.standard_normal(G4).astype(np.float32) * s
    Wh = rng.standard_normal((G4, H), dtype=np.float32) * s
    bh = rng.standard_normal(G4).astype(np.float32) * s

    nc = _get_program(t_steps)
    in_maps = _host_prep(x, Wx, bx, Wh, bh, t_steps)
    sim = CoreSim(nc, trace=False)
    for k, v in in_maps[0].items():
        sim.tensor(k)[:] = v
    sim.simulate()
    yh = _unshard_y(np.array(sim.tensor("y")), t_steps)  # [BL, t, H]
    ref = _np_lstm(x[:BL, :t_steps], Wx, bx, Wh, bh)
    err = np.abs(yh - ref)
    scale = np.abs(ref).max()
    print(f"selftest T={t_steps}: max abs err {err.max():.3e} (scale {scale:.3f})")
    return err.max()


if __name__ == "__main__":
    _selftest(16)


# revision 18
# speedup vs baseline: 1.0089x; 1.0089x over previous
"""BiLSTM Trainium2 kernel — transposed-domain recurrence.

Problem: B=32, T=512, I=512, H=512 bidirectional LSTM (torch gate order
i,f,g,o; shared Wx/Wh/bx/bh across directions; backward outputs stacked in
processing order).

Sharding: 8 cores = 2 directions x 4 batch groups of 8. Backward cores get
x time-reversed on the host so every core runs the same forward program.

Device program (per core, BL=8 batch rows):
  The whole recurrence runs transposed: gates live as gatesT [2048 gate dims
  (16 chunks of 128 partitions), 8 batch] in a single PSUM tile [128, 16*8].
  Each chunk accumulates 9 bf16 matmuls: 1 bias (K=1), 4 x-chunks and 4
  h-chunks, all with stationary 128x128 weight blocks and an 8-column moving
  operand (8 cycles each). The x/bias matmuls for step t+1 run during step
  t's epilogue, so only the 64 h-matmuls sit on the serial chain.

  Everything is sigmoid: the g gate's weights/bias are pre-scaled x2 on the
  host so tanh(a) = 2*sigmoid(2a)-1, and Wh is pre-doubled so the carried
  state is hhat = h/2. Epilogue per step (tiles [128, 32] = 4 h-chunks x 8):
      sgi  = sigmoid(gatesT[g,i])      ACT  [128, 64]
      sfo  = sigmoid(gatesT[f,o])      ACT  [128, 64]
      x1   = (ghat - 0.5) * i          DVE  scalar_tensor_tensor
      fc   = f * c                     Pool (parallel with x1)
      c    = 2*x1 + fc                 DVE  scalar_tensor_tensor
      chat = sigmoid(2c)               ACT  (tanh(c) = 2*chat - 1)
      hhat = (chat - 0.5) * o -> bf16  DVE  (= h/2; y = 2*hhat on host)
  hhat is written straight into a [128, 16*32] window buffer that is both
  the next steps' matmul rhs and the y DMA source (one DMA per 16 steps).
"""

import numpy as np

B, T, I, H = 32, 512, 512, 512
G4 = 4 * H            # 2048 gate width
BL = 8                # batch rows per core
NCH = 16              # gate chunks of 128
YW = 16               # steps per y DMA window
FILL_N = 0            # PE filler matmuls per step (keep max p-state)
FILL_AP = 256

_COMPILED = {}


def _build_program(t_steps: int):
    import concourse.bass as bass
    import concourse.tile as tile
    from concourse import bacc, mybir

    dt = mybir.dt
    f32 = dt.float32
    bf16 = dt.bfloat16
    alu = mybir.AluOpType
    sigf = mybir.ActivationFunctionType.Sigmoid

    nc = bacc.Bacc("TRN2", target_bir_lowering=False, debug=False)

    xT_d = nc.declare_dram_parameter("xT", [I, t_steps * BL], bf16, isOutput=False)
    WxT_d = nc.declare_dram_parameter("WxT", [I, G4], bf16, isOutput=False)
    WhT_d = nc.declare_dram_parameter("WhT", [H, G4], bf16, isOutput=False)
    bT_d = nc.declare_dram_parameter("bT", [1, G4], bf16, isOutput=False)
    y_d = nc.declare_dram_parameter("y", [128, t_steps * 4 * BL], bf16, isOutput=True)

    CW = 4 * BL  # 32 free columns per [128, 32] epilogue tile

    with tile.TileContext(nc) as tc:
        with (
            tc.tile_pool(name="const", bufs=1) as const_pool,
            tc.tile_pool(name="ybuf", bufs=2) as ybuf_pool,
            tc.tile_pool(name="sig", bufs=4) as sig_pool,
            tc.tile_pool(name="ep", bufs=4) as ep_pool,
            tc.tile_pool(name="gates", bufs=2, space="PSUM") as gates_pool,
        ):
            # ---- constants ----
            # Loads are spread over the SP/ACT/DVE DMA rings and ordered so
            # step 0's inputs (bT, wxT, first xT quarter) land first; whT is
            # only needed from step 1 and the later xT quarters much later.
            whT = [const_pool.tile([128, G4], bf16, tag=f"whT{k}", name=f"whT{k}")
                   for k in range(4)]
            wxT = [const_pool.tile([128, G4], bf16, tag=f"wxT{k}", name=f"wxT{k}")
                   for k in range(4)]
            xT = [const_pool.tile([128, t_steps * BL], bf16, tag=f"xT{k}", name=f"xT{k}")
                  for k in range(4)]
            bT = const_pool.tile([1, G4], bf16, tag="bT")
            nc.sync.dma_start(out=bT, in_=bT_d[:, :])
            for k in range(4):
                eng = nc.sync if k < 2 else nc.gpsimd
                eng.dma_start(out=wxT[k], in_=WxT_d[k * 128 : (k + 1) * 128, :])
            TQ = t_steps * BL // 4
            for k in range(4):
                nc.sync.dma_start(
                    out=xT[k][:, 0:TQ], in_=xT_d[k * 128 : (k + 1) * 128, 0:TQ]
                )
            for k in range(4):
                nc.gpsimd.dma_start(out=whT[k], in_=WhT_d[k * 128 : (k + 1) * 128, :])
            for q in range(1, 4):
                for k in range(4):
                    nc.gpsimd.dma_start(
                        out=xT[k][:, q * TQ : (q + 1) * TQ],
                        in_=xT_d[k * 128 : (k + 1) * 128, q * TQ : (q + 1) * TQ],
                    )
            ones = const_pool.tile([1, BL], bf16, tag="ones")
            nc.vector.memset(ones, 1.0)

            # chunk ch covers gate dims [ch*128, (ch+1)*128) in device order
            # [f(0..3) g(4..7) i(8..11) o(12..15)]
            def csl(ch):
                return slice(ch * BL, (ch + 1) * BL)

            # PSUM zero-region rule: one start and one stop per bank per
            # step. The first matmul of a step (bias ch 0) starts the group
            # (hardware zeroes the whole bank lazily); the last matmul of
            # the step stops it.
            def emit_bias_x(gates, t):
                stop = t == 0  # at t==0 there is no h part; stop here
                for ch in range(NCH):
                    out = gates[:, csl(ch)]
                    nc.tensor.matmul(
                        out,
                        lhsT=bT[:, ch * 128 : (ch + 1) * 128],
                        rhs=ones[:, :],
                        start=ch == 0,
                        stop=False,
                    )
                    for k in range(4):
                        nc.tensor.matmul(
                            out,
                            lhsT=wxT[k][:, ch * 128 : (ch + 1) * 128],
                            rhs=xT[k][:, t * BL : (t + 1) * BL],
                            start=False,
                            stop=stop and ch == NCH - 1 and k == 3,
                        )

            def emit_h(gates, hhat):
                for ch in range(NCH):
                    out = gates[:, csl(ch)]
                    for k in range(4):
                        nc.tensor.matmul(
                            out,
                            lhsT=whT[k][:, ch * 128 : (ch + 1) * 128],
                            rhs=hhat[:, k * BL : (k + 1) * BL],
                            start=False,
                            stop=ch == NCH - 1 and k == 3,
                        )

            # ---- prologue ----
            # filler matmuls keep the PE busy through the epilogue so the
            # tensor engine never drops out of its max p-state.
            fill_ps = gates_pool.tile([128, 512], f32, tag="fill", name="fill_ps")

            def emit_fillers(n, ap=256):
                ap = min(ap, t_steps * BL)
                for _ in range(n):
                    nc.tensor.matmul(
                        fill_ps[:, 0:ap],
                        lhsT=whT[0][:, 0:128],
                        rhs=xT[0][:, 0:ap],
                        start=True,
                        stop=True,
                    )

            c = ep_pool.tile([128, CW], f32, tag="c")
            nc.vector.memset(c, 0.0)

            # full-bank tiles (2KB/partition) so each step owns its zero regions
            gates = gates_pool.tile([128, 512], f32, tag="gates", name="gates0")
            emit_bias_x(gates, 0)

            ybuf = ybuf_pool.tile([128, YW * CW], bf16, tag="ybuf", name="ybuf0")
            hhat = None

            # ---- main loop ----
            for t in range(t_steps):
                if t > 0:
                    emit_h(gates, hhat)

                # queue next step's bias+x matmuls right behind this step's
                # h-mms, BEFORE emitting this step's ACT reads: the scheduler
                # binds the PSUM WAR dependency to the reads emitted so far
                # (step t-1's, long done), not step t's.
                if t + 1 < t_steps:
                    gates_next = gates_pool.tile(
                        [128, 512], f32, tag="gates", name=f"gates{t + 1}"
                    )
                    emit_bias_x(gates_next, t + 1)
                else:
                    gates_next = None
                emit_fillers(FILL_N, FILL_AP)

                # One ACT covers f,g,i (the chain-critical gates); o follows.
                # Consecutive same-engine ACTs get chained on each other's
                # completion sems by the scheduler, so fewer ACTs = less
                # serial latency.
                sfgi = sig_pool.tile([128, 3 * CW], f32, tag="sfgi", name=f"sfgi{t}")
                so = sig_pool.tile([128, CW], f32, tag="so", name=f"so{t}")
                nc.scalar.activation(sfgi, gates[:, 0 : 3 * CW], sigf)
                nc.scalar.activation(so, gates[:, 3 * CW : 4 * CW], sigf)

                x1 = ep_pool.tile([128, CW], f32, tag="x1")
                fc = ep_pool.tile([128, CW], f32, tag="fc")
                cn = ep_pool.tile([128, CW], f32, tag="c")
                chat = ep_pool.tile([128, CW], f32, tag="chat")
                # x1/fc/cn back-to-back on DVE: same-queue FIFO needs no sems
                # x1 = (ghat - 0.5) * i
                nc.vector.scalar_tensor_tensor(
                    x1, sfgi[:, CW : 2 * CW], 0.5, sfgi[:, 2 * CW : 3 * CW],
                    op0=alu.subtract, op1=alu.mult,
                )
                # fc = f * c
                nc.vector.tensor_tensor(fc, sfgi[:, 0:CW], c, op=alu.mult)
                # c = 2*x1 + fc
                nc.vector.scalar_tensor_tensor(
                    cn, x1, 2.0, fc, op0=alu.mult, op1=alu.add,
                )
                # chat = sigmoid(2c)      on ACT
                nc.scalar.activation(chat, cn, sigf, scale=2.0)
                # hhat = (chat-0.5) * o   on DVE, bf16, into the y window buf
                j = t % YW
                hn = ybuf[:, j * CW : (j + 1) * CW]
                nc.vector.scalar_tensor_tensor(
                    hn, chat, 0.5, so,
                    op0=alu.subtract, op1=alu.mult,
                )

                if j == YW - 1:
                    w = t // YW
                    nc.sync.dma_start(
                        out=y_d[:, w * YW * CW : (w + 1) * YW * CW], in_=ybuf
                    )
                    if t + 1 < t_steps:
                        ybuf = ybuf_pool.tile(
                            [128, YW * CW], bf16, tag="ybuf", name=f"ybuf{t + 1}"
                        )

                c = cn
                hhat = hn
                gates = gates_next

    nc.compile()
    return nc


def _get_program(t_steps: int):
    if t_steps not in _COMPILED:
        _COMPILED[t_steps] = _build_program(t_steps)
    return _COMPILED[t_steps]


# gate permutation: device order [f, g, i, o] from torch order [i, f, g, o]
_PERM = np.concatenate(
    [np.arange(512, 1024), np.arange(1024, 1536), np.arange(0, 512), np.arange(1536, 2048)]
)


def _host_prep(x, Wx, bx, Wh, bh, t_steps):
    import ml_dtypes

    bf = ml_dtypes.bfloat16
    Wx_p = Wx[_PERM].astype(np.float32).copy()
    Wh_p = Wh[_PERM].astype(np.float32).copy()
    b_p = (bx + bh)[_PERM].astype(np.float32).copy()
    # g rows (device chunks 4..7) carry 2x so sigmoid(2a) = (tanh(a)+1)/2
    Wx_p[512:1024] *= 2.0
    b_p[512:1024] *= 2.0
    Wh_p[512:1024] *= 2.0
    # carried state is hhat = h/2 -> double all Wh columns' effect
    Wh_p *= 2.0

    WxT = np.ascontiguousarray(Wx_p.T).astype(bf)
    WhT = np.ascontiguousarray(Wh_p.T).astype(bf)
    bT = np.ascontiguousarray(b_p.reshape(1, G4)).astype(bf)

    in_maps = []
    for core in range(8):
        d, g = divmod(core, 4)
        xc = x[g * BL : (g + 1) * BL, :t_steps]
        if d == 1:
            xc = xc[:, ::-1]
        xT = np.ascontiguousarray(xc.transpose(2, 1, 0).reshape(I, t_steps * BL))
        in_maps.append(
            {"xT": xT.astype(bf), "WxT": WxT, "WhT": WhT, "bT": bT}
        )
    return in_maps


def _unshard_y(y, t_steps):
    # y [128, t*4*BL] bf16 -> h [BL, t, H]; h = 2*hhat
    yh = 2.0 * np.asarray(y, dtype=np.float32).reshape(128, t_steps, 4, BL)
    return yh.transpose(3, 1, 2, 0).reshape(BL, t_steps, H)


def kernel(x, Wx, bx, Wh, bh):
    from concourse.bass_utils import run_bass_kernel_spmd

    x = np.asarray(x, dtype=np.float32)
    Wx = np.asarray(Wx, dtype=np.float32)
    bx = np.asarray(bx, dtype=np.float32)
    Wh = np.asarray(Wh, dtype=np.float32)
    bh = np.asarray(bh, dtype=np.float32)
    nc = _get_program(T)
    in_maps = _host_prep(x, Wx, bx, Wh, bh, T)
    res = run_bass_kernel_spmd(nc, in_maps, list(range(8)))
    out = np.empty((B, T, 2 * H), dtype=np.float32)
    for core in range(8):
        d, g = divmod(core, 4)
        yh = _unshard_y(res.results[core]["y"], T)
        out[g * BL : (g + 1) * BL, :, d * H : (d + 1) * H] = yh
    return out


def _np_lstm(x, Wx, bx, Wh, bh):
    """Single-direction numpy reference for self-test (forward order)."""
    b_, t_, _ = x.shape
    h = np.zeros((b_, H), np.float32)
    c = np.zeros((b_, H), np.float32)
    gx = x @ Wx.T + bx
    ys = []
    for t in range(t_):
        gates = gx[:, t] + h @ Wh.T + bh
        i_g, f_g, g_g, o_g = np.split(gates, 4, axis=1)
        i_t = 1 / (1 + np.exp(-i_g))
        f_t = 1 / (1 + np.exp(-f_g))
        g_t = np.tanh(g_g)
        o_t = 1 / (1 + np.exp(-o_g))
        c = c * f_t + i_t * g_t
        h = o_t * np.tanh(c)
        ys.append(h)
    return np.stack(ys, 1)


def _selftest(t_steps=16):
    from concourse.bass_interp import CoreSim

    rng = np.random.default_rng(0)
    s = 1.0 / np.sqrt(H)
    x = rng.standard_normal((B, T, I), dtype=np.float32)
    Wx = rng.standard_normal((G4, I), dtype=np.float32) * s
    bx = rng.standard_normal(G4).astype(np.float32) * s
    Wh = rng.standard_normal((G4, H), dtype=np.float32) * s
    bh = rng.standard_normal(G4).astype(np.float32) * s

    nc = _get_program(t_steps)
    in_maps = _host_prep(x, Wx, bx, Wh, bh, t_steps)
    sim = CoreSim(nc, trace=False)
    for k, v in in_maps[0].items():
        sim.tensor(k)[:] = v
    sim.simulate()
    yh = _unshard_y(np.array(sim.tensor("y")), t_steps)  # [BL, t, H]
    ref = _np_lstm(x[:BL, :t_steps], Wx, bx, Wh, bh)
    err = np.abs(yh - ref)
    scale = np.abs(ref).max()
    print(f"selftest T={t_steps}: max abs err {err.max():.3e} (scale {scale:.3f})")
    return err.max()


if __name__ == "__main__":
    _selftest(16)


# revision 19
# speedup vs baseline: 1.0104x; 1.0015x over previous
"""BiLSTM Trainium2 kernel — transposed-domain recurrence.

Problem: B=32, T=512, I=512, H=512 bidirectional LSTM (torch gate order
i,f,g,o; shared Wx/Wh/bx/bh across directions; backward outputs stacked in
processing order).

Sharding: 8 cores = 2 directions x 4 batch groups of 8. Backward cores get
x time-reversed on the host so every core runs the same forward program.

Device program (per core, BL=8 batch rows):
  The whole recurrence runs transposed: gates live as gatesT [2048 gate dims
  = 16 chunks of 128 partitions, 8 batch] in one PSUM bank [128, 16*8].
  Each chunk accumulates 9 bf16 matmuls: 1 bias (K=1), 4 x-blocks and 4
  h-blocks, all with stationary 128x128 weight blocks and an 8-column moving
  operand (8 cycles each at 2.4GHz). The bias/x matmuls for step t+1 are
  queued behind step t's h-matmuls and execute during t's epilogue, so only
  the 64 h-matmuls (~213ns) sit on the serial chain. One PSUM accumulation
  group per bank per step (start on the first bias matmul, stop on the last
  h-matmul) per the zero-region rule.

  Everything is sigmoid: the g gate's weights/bias are pre-scaled x2 on the
  host so tanh(a) = 2*sigmoid(2a)-1, and Wh is pre-doubled so the carried
  state is hhat = h/2. Epilogue per step (tiles [128, 32] = 4 h-chunks x 8,
  device chunk order f,g,i,o):
      sfgi = sigmoid(gatesT[f,g,i])    ACT [128, 96] (one op: consecutive
                                       ACTs chain on completion sems)
      so   = sigmoid(gatesT[o])        ACT [128, 32] (off critical path)
      x1   = (ghat - 0.5) * i          DVE scalar_tensor_tensor
      fc   = f * c                     DVE (back-to-back, no cross sems)
      c    = 2*x1 + fc                 DVE scalar_tensor_tensor
      chat = sigmoid(2c)               ACT (tanh(c) = 2*chat - 1)
      hhat = (chat - 0.5) * o -> bf16  DVE (= h/2; y = 2*hhat on host)
  hhat lands directly in a [128, 16*32] bf16 window buffer that is both the
  next steps' matmul rhs and the y DMA source (one DMA per 16 steps).

  Steady state is ~2053 ns/step, dominated by fixed cross-engine latencies
  (PE psum-ack 173, ACT access-ack ~370x2, DVE write-acks, sem hops).
  Prologue DMAs are split across the SP HWDGE ring (step-0-critical: bias,
  WxT, first xT quarter) and the SWDGE ring (rest) to overlap transfers
  with the first steps.
"""

import numpy as np

B, T, I, H = 32, 512, 512, 512
G4 = 4 * H            # 2048 gate width
BL = 8                # batch rows per core
NCH = 16              # gate chunks of 128
YW = 16               # steps per y DMA window

_COMPILED = {}


def _build_program(t_steps: int):
    import concourse.bass as bass
    import concourse.tile as tile
    from concourse import bacc, mybir

    dt = mybir.dt
    f32 = dt.float32
    bf16 = dt.bfloat16
    alu = mybir.AluOpType
    sigf = mybir.ActivationFunctionType.Sigmoid

    nc = bacc.Bacc("TRN2", target_bir_lowering=False, debug=False)

    xT_d = nc.declare_dram_parameter("xT", [I, t_steps * BL], bf16, isOutput=False)
    WxT_d = nc.declare_dram_parameter("WxT", [I, G4], bf16, isOutput=False)
    WhT_d = nc.declare_dram_parameter("WhT", [H, G4], bf16, isOutput=False)
    bT_d = nc.declare_dram_parameter("bT", [1, G4], bf16, isOutput=False)
    y_d = nc.declare_dram_parameter("y", [128, t_steps * 4 * BL], bf16, isOutput=True)

    CW = 4 * BL  # 32 free columns per [128, 32] epilogue tile

    with tile.TileContext(nc) as tc:
        with (
            tc.tile_pool(name="const", bufs=1) as const_pool,
            tc.tile_pool(name="ybuf", bufs=2) as ybuf_pool,
            tc.tile_pool(name="sig", bufs=4) as sig_pool,
            tc.tile_pool(name="ep", bufs=4) as ep_pool,
            tc.tile_pool(name="gates", bufs=2, space="PSUM") as gates_pool,
        ):
            # ---- constants ----
            # Loads are spread over the SP/ACT/DVE DMA rings and ordered so
            # step 0's inputs (bT, wxT, first xT quarter) land first; whT is
            # only needed from step 1 and the later xT quarters much later.
            whT = [const_pool.tile([128, G4], bf16, tag=f"whT{k}", name=f"whT{k}")
                   for k in range(4)]
            wxT = [const_pool.tile([128, G4], bf16, tag=f"wxT{k}", name=f"wxT{k}")
                   for k in range(4)]
            xT = [const_pool.tile([128, t_steps * BL], bf16, tag=f"xT{k}", name=f"xT{k}")
                  for k in range(4)]
            bT = const_pool.tile([1, G4], bf16, tag="bT")
            nc.sync.dma_start(out=bT, in_=bT_d[:, :])
            for k in range(4):
                eng = nc.sync if k < 2 else nc.gpsimd
                eng.dma_start(out=wxT[k], in_=WxT_d[k * 128 : (k + 1) * 128, :])
            TQ = t_steps * BL // 4
            for k in range(4):
                nc.sync.dma_start(
                    out=xT[k][:, 0:TQ], in_=xT_d[k * 128 : (k + 1) * 128, 0:TQ]
                )
            for k in range(4):
                nc.gpsimd.dma_start(out=whT[k], in_=WhT_d[k * 128 : (k + 1) * 128, :])
            for q in range(1, 4):
                for k in range(4):
                    nc.gpsimd.dma_start(
                        out=xT[k][:, q * TQ : (q + 1) * TQ],
                        in_=xT_d[k * 128 : (k + 1) * 128, q * TQ : (q + 1) * TQ],
                    )
            ones = const_pool.tile([1, BL], bf16, tag="ones")
            nc.vector.memset(ones, 1.0)

            # chunk ch covers gate dims [ch*128, (ch+1)*128) in device order
            # [f(0..3) g(4..7) i(8..11) o(12..15)]
            def csl(ch):
                return slice(ch * BL, (ch + 1) * BL)

            # PSUM zero-region rule: one start and one stop per bank per
            # step. The first matmul of a step (bias ch 0) starts the group
            # (hardware zeroes the whole bank lazily); the last matmul of
            # the step stops it.
            def emit_bias_x(gates, t):
                stop = t == 0  # at t==0 there is no h part; stop here
                for ch in range(NCH):
                    out = gates[:, csl(ch)]
                    nc.tensor.matmul(
                        out,
                        lhsT=bT[:, ch * 128 : (ch + 1) * 128],
                        rhs=ones[:, :],
                        start=ch == 0,
                        stop=False,
                    )
                    for k in range(4):
                        nc.tensor.matmul(
                            out,
                            lhsT=wxT[k][:, ch * 128 : (ch + 1) * 128],
                            rhs=xT[k][:, t * BL : (t + 1) * BL],
                            start=False,
                            stop=stop and ch == NCH - 1 and k == 3,
                        )

            def emit_h(gates, hhat):
                for ch in range(NCH):
                    out = gates[:, csl(ch)]
                    for k in range(4):
                        nc.tensor.matmul(
                            out,
                            lhsT=whT[k][:, ch * 128 : (ch + 1) * 128],
                            rhs=hhat[:, k * BL : (k + 1) * BL],
                            start=False,
                            stop=ch == NCH - 1 and k == 3,
                        )

            # ---- prologue ----
            c = ep_pool.tile([128, CW], f32, tag="c")
            nc.vector.memset(c, 0.0)

            # full-bank tiles (2KB/partition) so each step owns its zero regions
            gates = gates_pool.tile([128, 512], f32, tag="gates", name="gates0")
            emit_bias_x(gates, 0)

            ybuf = ybuf_pool.tile([128, YW * CW], bf16, tag="ybuf", name="ybuf0")
            hhat = None

            # ---- main loop ----
            for t in range(t_steps):
                if t > 0:
                    emit_h(gates, hhat)

                # queue next step's bias+x matmuls right behind this step's
                # h-mms, BEFORE emitting this step's ACT reads: the scheduler
                # binds the PSUM WAR dependency to the reads emitted so far
                # (step t-1's, long done), not step t's.
                if t + 1 < t_steps:
                    gates_next = gates_pool.tile(
                        [128, 512], f32, tag="gates", name=f"gates{t + 1}"
                    )
                    emit_bias_x(gates_next, t + 1)
                else:
                    gates_next = None

                # One ACT covers f,g,i (the chain-critical gates); o follows.
                # Consecutive same-engine ACTs get chained on each other's
                # completion sems by the scheduler, so fewer ACTs = less
                # serial latency.
                sfgi = sig_pool.tile([128, 3 * CW], f32, tag="sfgi", name=f"sfgi{t}")
                so = sig_pool.tile([128, CW], f32, tag="so", name=f"so{t}")
                nc.scalar.activation(sfgi, gates[:, 0 : 3 * CW], sigf)
                nc.scalar.activation(so, gates[:, 3 * CW : 4 * CW], sigf)

                x1 = ep_pool.tile([128, CW], f32, tag="x1")
                fc = ep_pool.tile([128, CW], f32, tag="fc")
                cn = ep_pool.tile([128, CW], f32, tag="c")
                chat = ep_pool.tile([128, CW], f32, tag="chat")
                # x1/fc/cn back-to-back on DVE: same-queue FIFO needs no sems
                # x1 = (ghat - 0.5) * i
                nc.vector.scalar_tensor_tensor(
                    x1, sfgi[:, CW : 2 * CW], 0.5, sfgi[:, 2 * CW : 3 * CW],
                    op0=alu.subtract, op1=alu.mult,
                )
                # fc = f * c
                nc.vector.tensor_tensor(fc, sfgi[:, 0:CW], c, op=alu.mult)
                # c = 2*x1 + fc
                nc.vector.scalar_tensor_tensor(
                    cn, x1, 2.0, fc, op0=alu.mult, op1=alu.add,
                )
                # chat = sigmoid(2c)      on ACT
                nc.scalar.activation(chat, cn, sigf, scale=2.0)
                # hhat = (chat-0.5) * o   on DVE, bf16, into the y window buf
                j = t % YW
                hn = ybuf[:, j * CW : (j + 1) * CW]
                nc.vector.scalar_tensor_tensor(
                    hn, chat, 0.5, so,
                    op0=alu.subtract, op1=alu.mult,
                )

                if j == YW - 1:
                    w = t // YW
                    nc.sync.dma_start(
                        out=y_d[:, w * YW * CW : (w + 1) * YW * CW], in_=ybuf
                    )
                    if t + 1 < t_steps:
                        ybuf = ybuf_pool.tile(
                            [128, YW * CW], bf16, tag="ybuf", name=f"ybuf{t + 1}"
                        )

                c = cn
                hhat = hn
                gates = gates_next

    nc.compile()
    return nc


def _get_program(t_steps: int):
    if t_steps not in _COMPILED:
        _COMPILED[t_steps] = _build_program(t_steps)
    return _COMPILED[t_steps]


# gate permutation: device order [f, g, i, o] from torch order [i, f, g, o]
_PERM = np.concatenate(
    [np.arange(512, 1024), np.arange(1024, 1536), np.arange(0, 512), np.arange(1536, 2048)]
)


def _host_prep(x, Wx, bx, Wh, bh, t_steps):
    import ml_dtypes

    bf = ml_dtypes.bfloat16
    Wx_p = Wx[_PERM].astype(np.float32).copy()
    Wh_p = Wh[_PERM].astype(np.float32).copy()
    b_p = (bx + bh)[_PERM].astype(np.float32).copy()
    # g rows (device chunks 4..7) carry 2x so sigmoid(2a) = (tanh(a)+1)/2
    Wx_p[512:1024] *= 2.0
    b_p[512:1024] *= 2.0
    Wh_p[512:1024] *= 2.0
    # carried state is hhat = h/2 -> double all Wh columns' effect
    Wh_p *= 2.0

    WxT = np.ascontiguousarray(Wx_p.T).astype(bf)
    WhT = np.ascontiguousarray(Wh_p.T).astype(bf)
    bT = np.ascontiguousarray(b_p.reshape(1, G4)).astype(bf)

    in_maps = []
    for core in range(8):
        d, g = divmod(core, 4)
        xc = x[g * BL : (g + 1) * BL, :t_steps]
        if d == 1:
            xc = xc[:, ::-1]
        xT = np.ascontiguousarray(xc.transpose(2, 1, 0).reshape(I, t_steps * BL))
        in_maps.append(
            {"xT": xT.astype(bf), "WxT": WxT, "WhT": WhT, "bT": bT}
        )
    return in_maps


def _unshard_y(y, t_steps):
    # y [128, t*4*BL] bf16 -> h [BL, t, H]; h = 2*hhat
    yh = 2.0 * np.asarray(y, dtype=np.float32).reshape(128, t_steps, 4, BL)
    return yh.transpose(3, 1, 2, 0).reshape(BL, t_steps, H)


def kernel(x, Wx, bx, Wh, bh):
    from concourse.bass_utils import run_bass_kernel_spmd

    x = np.asarray(x, dtype=np.float32)
    Wx = np.asarray(Wx, dtype=np.float32)
    bx = np.asarray(bx, dtype=np.float32)
    Wh = np.asarray(Wh, dtype=np.float32)
    bh = np.asarray(bh, dtype=np.float32)
    nc = _get_program(T)
    in_maps = _host_prep(x, Wx, bx, Wh, bh, T)
    res = run_bass_kernel_spmd(nc, in_maps, list(range(8)))
    out = np.empty((B, T, 2 * H), dtype=np.float32)
    for core in range(8):
        d, g = divmod(core, 4)
        yh = _unshard_y(res.results[core]["y"], T)
        out[g * BL : (g + 1) * BL, :, d * H : (d + 1) * H] = yh
    return out


def _np_lstm(x, Wx, bx, Wh, bh):
    """Single-direction numpy reference for self-test (forward order)."""
    b_, t_, _ = x.shape
    h = np.zeros((b_, H), np.float32)
    c = np.zeros((b_, H), np.float32)
    gx = x @ Wx.T + bx
    ys = []
    for t in range(t_):
        gates = gx[:, t] + h @ Wh.T + bh
        i_g, f_g, g_g, o_g = np.split(gates, 4, axis=1)
        i_t = 1 / (1 + np.exp(-i_g))
        f_t = 1 / (1 + np.exp(-f_g))
        g_t = np.tanh(g_g)
        o_t = 1 / (1 + np.exp(-o_g))
        c = c * f_t + i_t * g_t
        h = o_t * np.tanh(c)
        ys.append(h)
    return np.stack(ys, 1)


def _selftest(t_steps=16):
    from concourse.bass_interp import CoreSim

    rng = np.random.default_rng(0)
    s = 1.0 / np.sqrt(H)
    x = rng.standard_normal((B, T, I), dtype=np.float32)
    Wx = rng.standard_normal((G4, I), dtype=np.float32) * s
    bx = rng.standard_normal(G4).astype(np.float32) * s
    Wh = rng.standard_normal((G4, H), dtype=np.float32) * s
    bh = rng.standard_normal(G4).astype(np.float32) * s

    nc = _get_program(t_steps)
    in_maps = _host_prep(x, Wx, bx, Wh, bh, t_steps)
    sim = CoreSim(nc, trace=False)
    for k, v in in_maps[0].items():
        sim.tensor(k)[:] = v
    sim.simulate()
    yh = _unshard_y(np.array(sim.tensor("y")), t_steps)  # [BL, t, H]
    ref = _np_lstm(x[:BL, :t_steps], Wx, bx, Wh, bh)
    err = np.abs(yh - ref)
    scale = np.abs(ref).max()
    print(f"selftest T={t_steps}: max abs err {err.max():.3e} (scale {scale:.3f})")
    return err.max()


if __name__ == "__main__":
    _selftest(16)


# revision 23
# speedup vs baseline: 1.0127x; 1.0022x over previous
"""BiLSTM Trainium2 kernel — transposed-domain recurrence.

Problem: B=32, T=512, I=512, H=512 bidirectional LSTM (torch gate order
i,f,g,o; shared Wx/Wh/bx/bh across directions; backward outputs stacked in
processing order).

Sharding: 8 cores = 2 directions x 4 batch groups of 8. Backward cores get
x time-reversed on the host so every core runs the same forward program.

Device program (per core, BL=8 batch rows):
  The whole recurrence runs transposed: gates live as gatesT [2048 gate dims
  = 16 chunks of 128 partitions, 8 batch] in one PSUM bank [128, 16*8].
  Each chunk accumulates 9 bf16 matmuls: 1 bias (K=1), 4 x-blocks and 4
  h-blocks, all with stationary 128x128 weight blocks and an 8-column moving
  operand (8 cycles each at 2.4GHz). The bias/x matmuls for step t+1 are
  queued behind step t's h-matmuls and execute during t's epilogue, so only
  the 64 h-matmuls (~213ns) sit on the serial chain. One PSUM accumulation
  group per bank per step (start on the first bias matmul, stop on the last
  h-matmul) per the zero-region rule.

  Everything is sigmoid: the g gate's weights/bias are pre-scaled x2 on the
  host so tanh(a) = 2*sigmoid(2a)-1, and Wh is pre-doubled so the carried
  state is hhat = h/2. Epilogue per step (tiles [128, 32] = 4 h-chunks x 8,
  device chunk order f,g,i,o):
      sfgi = sigmoid(gatesT[f,g,i])    ACT [128, 96] (one op: consecutive
                                       ACTs chain on completion sems)
      so   = sigmoid(gatesT[o])        ACT [128, 32] (off critical path)
      x1   = (ghat - 0.5) * i          DVE scalar_tensor_tensor
      fc   = f * c                     DVE (back-to-back, no cross sems)
      c    = 2*x1 + fc                 DVE scalar_tensor_tensor
      chat = sigmoid(2c)               ACT (tanh(c) = 2*chat - 1)
      hhat = (chat - 0.5) * o -> bf16  DVE (= h/2; y = 2*hhat on host)
  hhat lands directly in a [128, 16*32] bf16 window buffer that is both the
  next steps' matmul rhs and the y DMA source (one DMA per 16 steps).

  Steady state is ~2053 ns/step, dominated by fixed cross-engine latencies
  (PE psum-ack 173, ACT access-ack ~370x2, DVE write-acks, sem hops).
  Prologue DMAs are split across the SP HWDGE ring (step-0-critical: bias,
  WxT, first xT quarter) and the SWDGE ring (rest) to overlap transfers
  with the first steps.
"""

import numpy as np

B, T, I, H = 32, 512, 512, 512
G4 = 4 * H            # 2048 gate width
BL = 8                # batch rows per core
NCH = 16              # gate chunks of 128
YW = 16               # steps per y DMA window

_COMPILED = {}


def _build_program(t_steps: int):
    import concourse.bass as bass
    import concourse.tile as tile
    from concourse import bacc, mybir

    dt = mybir.dt
    f32 = dt.float32
    bf16 = dt.bfloat16
    alu = mybir.AluOpType
    sigf = mybir.ActivationFunctionType.Sigmoid

    nc = bacc.Bacc("TRN2", target_bir_lowering=False, debug=False)

    xT_d = nc.declare_dram_parameter("xT", [I, t_steps * BL], bf16, isOutput=False)
    WxT_d = nc.declare_dram_parameter("WxT", [I, G4], bf16, isOutput=False)
    WhT_d = nc.declare_dram_parameter("WhT", [H, G4], bf16, isOutput=False)
    bT_d = nc.declare_dram_parameter("bT", [1, G4], bf16, isOutput=False)
    y_d = nc.declare_dram_parameter("y", [128, t_steps * 4 * BL], bf16, isOutput=True)

    CW = 4 * BL  # 32 free columns per [128, 32] epilogue tile

    with tile.TileContext(nc) as tc:
        with (
            tc.tile_pool(name="const", bufs=1) as const_pool,
            tc.tile_pool(name="ybuf", bufs=2) as ybuf_pool,
            tc.tile_pool(name="sig", bufs=4) as sig_pool,
            tc.tile_pool(name="ep", bufs=4) as ep_pool,
            tc.tile_pool(name="gates", bufs=2, space="PSUM") as gates_pool,
        ):
            # ---- constants ----
            # Loads are spread over the SP/ACT/DVE DMA rings and ordered so
            # step 0's inputs (bT, wxT, first xT quarter) land first; whT is
            # only needed from step 1 and the later xT quarters much later.
            whT = [const_pool.tile([128, G4], bf16, tag=f"whT{k}", name=f"whT{k}")
                   for k in range(4)]
            wxT = [const_pool.tile([128, G4], bf16, tag=f"wxT{k}", name=f"wxT{k}")
                   for k in range(4)]
            xT = [const_pool.tile([128, t_steps * BL], bf16, tag=f"xT{k}", name=f"xT{k}")
                  for k in range(4)]
            bT = const_pool.tile([1, G4], bf16, tag="bT")
            nc.sync.dma_start(out=bT, in_=bT_d[:, :])
            for k in range(4):
                eng = nc.sync if k < 2 else nc.gpsimd
                eng.dma_start(out=wxT[k], in_=WxT_d[k * 128 : (k + 1) * 128, :])
            TS0 = min(32, t_steps) * BL
            for k in range(4):
                nc.sync.dma_start(
                    out=xT[k][:, 0:TS0], in_=xT_d[k * 128 : (k + 1) * 128, 0:TS0]
                )
            for k in range(4):
                nc.gpsimd.dma_start(out=whT[k], in_=WhT_d[k * 128 : (k + 1) * 128, :])
            TQ = t_steps * BL // 4
            for q in range(4):
                for k in range(4):
                    q0 = max(q * TQ, TS0)
                    q1 = (q + 1) * TQ
                    if q1 > q0:
                        nc.gpsimd.dma_start(
                            out=xT[k][:, q0:q1],
                            in_=xT_d[k * 128 : (k + 1) * 128, q0:q1],
                        )
            ones = const_pool.tile([1, BL], bf16, tag="ones")
            nc.vector.memset(ones, 1.0)

            # chunk ch covers gate dims [ch*128, (ch+1)*128) in device order
            # [f(0..3) g(4..7) i(8..11) o(12..15)]
            def csl(ch):
                return slice(ch * BL, (ch + 1) * BL)

            # PSUM zero-region rule: one start and one stop per bank per
            # step. The first matmul of a step (bias ch 0) starts the group
            # (hardware zeroes the whole bank lazily); the last matmul of
            # the step stops it.
            def emit_bias_x(gates, t):
                stop = t == 0  # at t==0 there is no h part; stop here
                for ch in range(NCH):
                    out = gates[:, csl(ch)]
                    nc.tensor.matmul(
                        out,
                        lhsT=bT[:, ch * 128 : (ch + 1) * 128],
                        rhs=ones[:, :],
                        start=ch == 0,
                        stop=False,
                    )
                    for k in range(4):
                        nc.tensor.matmul(
                            out,
                            lhsT=wxT[k][:, ch * 128 : (ch + 1) * 128],
                            rhs=xT[k][:, t * BL : (t + 1) * BL],
                            start=False,
                            stop=stop and ch == NCH - 1 and k == 3,
                        )

            def emit_h(gates, hhat):
                # k-major so step 1 can begin as each whT k-chunk DMA lands
                for k in range(4):
                    for ch in range(NCH):
                        nc.tensor.matmul(
                            gates[:, csl(ch)],
                            lhsT=whT[k][:, ch * 128 : (ch + 1) * 128],
                            rhs=hhat[:, k * BL : (k + 1) * BL],
                            start=False,
                            stop=ch == NCH - 1 and k == 3,
                        )

            # ---- prologue ----
            c = ep_pool.tile([128, CW], f32, tag="c")
            nc.vector.memset(c, 0.0)

            # full-bank tiles (2KB/partition) so each step owns its zero regions
            gates = gates_pool.tile([128, 512], f32, tag="gates", name="gates0")
            emit_bias_x(gates, 0)

            ybuf = ybuf_pool.tile([128, YW * CW], bf16, tag="ybuf", name="ybuf0")
            hhat = None

            # ---- main loop ----
            for t in range(t_steps):
                if t > 0:
                    emit_h(gates, hhat)

                # queue next step's bias+x matmuls right behind this step's
                # h-mms, BEFORE emitting this step's ACT reads: the scheduler
                # binds the PSUM WAR dependency to the reads emitted so far
                # (step t-1's, long done), not step t's.
                if t + 1 < t_steps:
                    gates_next = gates_pool.tile(
                        [128, 512], f32, tag="gates", name=f"gates{t + 1}"
                    )
                    emit_bias_x(gates_next, t + 1)
                else:
                    gates_next = None

                # One ACT covers f,g,i (the chain-critical gates); o follows.
                # Consecutive same-engine ACTs get chained on each other's
                # completion sems by the scheduler, so fewer ACTs = less
                # serial latency.
                sfgi = sig_pool.tile([128, 3 * CW], f32, tag="sfgi", name=f"sfgi{t}")
                so = sig_pool.tile([128, CW], f32, tag="so", name=f"so{t}")
                nc.scalar.activation(sfgi, gates[:, 0 : 3 * CW], sigf)
                nc.scalar.activation(so, gates[:, 3 * CW : 4 * CW], sigf)

                x1 = ep_pool.tile([128, CW], f32, tag="x1")
                fc = ep_pool.tile([128, CW], f32, tag="fc")
                cn = ep_pool.tile([128, CW], f32, tag="c")
                chat = ep_pool.tile([128, CW], f32, tag="chat")
                # x1/fc/cn back-to-back on DVE: same-queue FIFO needs no sems
                # x1 = (ghat - 0.5) * i
                nc.vector.scalar_tensor_tensor(
                    x1, sfgi[:, CW : 2 * CW], 0.5, sfgi[:, 2 * CW : 3 * CW],
                    op0=alu.subtract, op1=alu.mult,
                )
                # fc = f * c
                nc.vector.tensor_tensor(fc, sfgi[:, 0:CW], c, op=alu.mult)
                # c = 2*x1 + fc
                nc.vector.scalar_tensor_tensor(
                    cn, x1, 2.0, fc, op0=alu.mult, op1=alu.add,
                )
                # chat = sigmoid(2c)      on ACT
                nc.scalar.activation(chat, cn, sigf, scale=2.0)
                # hhat = (chat-0.5) * o   on DVE, bf16, into the y window buf
                j = t % YW
                hn = ybuf[:, j * CW : (j + 1) * CW]
                nc.vector.scalar_tensor_tensor(
                    hn, chat, 0.5, so,
                    op0=alu.subtract, op1=alu.mult,
                )

                if j == YW - 1:
                    w = t // YW
                    nc.sync.dma_start(
                        out=y_d[:, w * YW * CW : (w + 1) * YW * CW], in_=ybuf
                    )
                    if t + 1 < t_steps:
                        ybuf = ybuf_pool.tile(
                            [128, YW * CW], bf16, tag="ybuf", name=f"ybuf{t + 1}"
                        )

                c = cn
                hhat = hn
                gates = gates_next

    nc.compile()
    return nc


def _get_program(t_steps: int):
    if t_steps not in _COMPILED:
        _COMPILED[t_steps] = _build_program(t_steps)
    return _COMPILED[t_steps]


# gate permutation: device order [f, g, i, o] from torch order [i, f, g, o]
_PERM = np.concatenate(
    [np.arange(512, 1024), np.arange(1024, 1536), np.arange(0, 512), np.arange(1536, 2048)]
)


def _host_prep(x, Wx, bx, Wh, bh, t_steps):
    import ml_dtypes

    bf = ml_dtypes.bfloat16
    Wx_p = Wx[_PERM].astype(np.float32).copy()
    Wh_p = Wh[_PERM].astype(np.float32).copy()
    b_p = (bx + bh)[_PERM].astype(np.float32).copy()
    # g rows (device chunks 4..7) carry 2x so sigmoid(2a) = (tanh(a)+1)/2
    Wx_p[512:1024] *= 2.0
    b_p[512:1024] *= 2.0
    Wh_p[512:1024] *= 2.0
    # carried state is hhat = h/2 -> double all Wh columns' effect
    Wh_p *= 2.0

    WxT = np.ascontiguousarray(Wx_p.T).astype(bf)
    WhT = np.ascontiguousarray(Wh_p.T).astype(bf)
    bT = np.ascontiguousarray(b_p.reshape(1, G4)).astype(bf)

    in_maps = []
    for core in range(8):
        d, g = divmod(core, 4)
        xc = x[g * BL : (g + 1) * BL, :t_steps]
        if d == 1:
            xc = xc[:, ::-1]
        xT = np.ascontiguousarray(xc.transpose(2, 1, 0).reshape(I, t_steps * BL))
        in_maps.append(
            {"xT": xT.astype(bf), "WxT": WxT, "WhT": WhT, "bT": bT}
        )
    return in_maps


def _unshard_y(y, t_steps):
    # y [128, t*4*BL] bf16 -> h [BL, t, H]; h = 2*hhat
    yh = 2.0 * np.asarray(y, dtype=np.float32).reshape(128, t_steps, 4, BL)
    return yh.transpose(3, 1, 2, 0).reshape(BL, t_steps, H)


def kernel(x, Wx, bx, Wh, bh):
    from concourse.bass_utils import run_bass_kernel_spmd

    x = np.asarray(x, dtype=np.float32)
    Wx = np.asarray(Wx, dtype=np.float32)
    bx = np.asarray(bx, dtype=np.float32)
    Wh = np.asarray(Wh, dtype=np.float32)
    bh = np.asarray(bh, dtype=np.float32)
    nc = _get_program(T)
    in_maps = _host_prep(x, Wx, bx, Wh, bh, T)
    res = run_bass_kernel_spmd(nc, in_maps, list(range(8)))
    out = np.empty((B, T, 2 * H), dtype=np.float32)
    for core in range(8):
        d, g = divmod(core, 4)
        yh = _unshard_y(res.results[core]["y"], T)
        out[g * BL : (g + 1) * BL, :, d * H : (d + 1) * H] = yh
    return out


def _np_lstm(x, Wx, bx, Wh, bh):
    """Single-direction numpy reference for self-test (forward order)."""
    b_, t_, _ = x.shape
    h = np.zeros((b_, H), np.float32)
    c = np.zeros((b_, H), np.float32)
    gx = x @ Wx.T + bx
    ys = []
    for t in range(t_):
        gates = gx[:, t] + h @ Wh.T + bh
        i_g, f_g, g_g, o_g = np.split(gates, 4, axis=1)
        i_t = 1 / (1 + np.exp(-i_g))
        f_t = 1 / (1 + np.exp(-f_g))
        g_t = np.tanh(g_g)
        o_t = 1 / (1 + np.exp(-o_g))
        c = c * f_t + i_t * g_t
        h = o_t * np.tanh(c)
        ys.append(h)
    return np.stack(ys, 1)


def _selftest(t_steps=16):
    from concourse.bass_interp import CoreSim

    rng = np.random.default_rng(0)
    s = 1.0 / np.sqrt(H)
    x = rng.standard_normal((B, T, I), dtype=np.float32)
    Wx = rng.standard_normal((G4, I), dtype=np.float32) * s
    bx = rng.standard_normal(G4).astype(np.float32) * s
    Wh = rng.standard_normal((G4, H), dtype=np.float32) * s
    bh = rng.standard_normal(G4).astype(np.float32) * s

    nc = _get_program(t_steps)
    in_maps = _host_prep(x, Wx, bx, Wh, bh, t_steps)
    sim = CoreSim(nc, trace=False)
    for k, v in in_maps[0].items():
        sim.tensor(k)[:] = v
    sim.simulate()
    yh = _unshard_y(np.array(sim.tensor("y")), t_steps)  # [BL, t, H]
    ref = _np_lstm(x[:BL, :t_steps], Wx, bx, Wh, bh)
    err = np.abs(yh - ref)
    scale = np.abs(ref).max()
    print(f"selftest T={t_steps}: max abs err {err.max():.3e} (scale {scale:.3f})")
    return err.max()


if __name__ == "__main__":
    _selftest(16)
